# revision 1
# baseline (speedup 1.0000x reference)
"""Causal linear attention (elu+1 feature map) for Trainium2, 8 NeuronCores.

Problem: B=2, S=2048, D=1024, H=16, HD=64.
  q/k/v projections [S,D]@[D,H*HD], phi = elu+1, causal linear attention
  out[t] = (sum_{i<=t} (phi_q[t].phi_k[i]) v[i]) / (phi_q[t].sum_{i<=t} phi_k[i] + eps)

Sharding: core c -> (batch b=c//4, heads h0=4*(c%4) .. h0+3). No cross-core comm.
Host feeds x^T [D,S] per core (layout choice for the NEFF inputs) so the
contraction dim d sits on SBUF partitions with no on-chip transposes.

Device algorithm (per core, 4 heads, all fp32):
  - proj q,k -> phi_qT/phi_kT [64,2048] per head (head-pairs packed on 128 parts)
  - phi_k seq layout via PE transpose of phi_kT
  - v projected seq-major with an appended ones column (v_aug [128,65] per chunk)
  - chunked attention, L=128: A_T = phi_kT_c^T-free matmul -> mask (j<=t) ->
    out_psum = tril(A)^T-form matmul @ v_aug + phi_q_c @ S_prev ; S += phi_k_c^T @ v_aug
    The ones column of v_aug makes column 64 of out_psum the normalizer.
"""

import os
import threading

import numpy as np

B, S, D, H, HD = 2, 2048, 1024, 16, 64
EPS = 1e-6
N_CORES = 8
HPC = 4            # heads per core
HDC = HPC * HD     # 256 projected cols per core
NCHUNK = S // 128  # 16
F32 = None         # set after import

_lock = threading.Lock()
_cache = {}


def _build_nc(dump=False):
    import concourse.bass as bass
    import concourse.tile as tile
    from concourse import bacc, mybir

    f32 = mybir.dt.float32
    Alu = mybir.AluOpType
    Act = mybir.ActivationFunctionType

    nc = bacc.Bacc("TRN2", target_bir_lowering=False, debug=False)
    f32r = mybir.dt.float32r

    def R(ap):
        # fp32 data reinterpreted as float32r: full-rate PE streaming
        return ap.bitcast(f32r)

    xqT = nc.dram_tensor("xqT", [D, S], f32r, kind="ExternalInput").ap()
    xkT = nc.dram_tensor("xkT", [D, S], f32r, kind="ExternalInput").ap()
    xvT = nc.dram_tensor("xvT", [D, S], f32r, kind="ExternalInput").ap()
    wq = nc.dram_tensor("wq", [D, HDC], f32r, kind="ExternalInput").ap()
    wk = nc.dram_tensor("wk", [D, HDC], f32r, kind="ExternalInput").ap()
    wv = nc.dram_tensor("wv", [D, HDC], f32r, kind="ExternalInput").ap()
    out = nc.dram_tensor("out", [S, HDC], f32, kind="ExternalOutput").ap()
    if dump:
        d_phi_qT = [nc.dram_tensor(f"d_phi_qT{i}", [128, S], f32, kind="ExternalOutput").ap() for i in range(2)]
        d_phi_kT = [nc.dram_tensor(f"d_phi_kT{i}", [128, S], f32, kind="ExternalOutput").ap() for i in range(2)]
        d_phi_ks = nc.dram_tensor("d_phi_ks", [128, NCHUNK * HDC], f32, kind="ExternalOutput").ap()
        d_v_aug = nc.dram_tensor("d_v_aug", [128, NCHUNK * HPC * 65], f32, kind="ExternalOutput").ap()

    DC = D // 128  # 8 contraction chunks

    with tile.TileContext(nc) as tc:
        with (
            tc.tile_pool(name="consts", bufs=1) as consts,
            tc.tile_pool(name="weights", bufs=1) as wpool,
            tc.tile_pool(name="resident", bufs=1) as res,
            tc.tile_pool(name="xin", bufs=30) as xin,
            tc.tile_pool(name="work", bufs=3) as work,
            tc.tile_pool(name="attn", bufs=3) as attn,
            tc.tile_pool(name="psum", bufs=2, space="PSUM") as psum,
        ):
            # ---- constants ----
            ones = consts.tile([128, 128], f32)
            nc.vector.memset(ones[:], 1.0)
            # identity (two 64x64 diagonal blocks are slices of the 128x128 I)
            ident = consts.tile([128, 128], f32)
            nc.gpsimd.affine_select(
                ident[:], ones[:], pattern=[[-1, 128]], base=0,
                channel_multiplier=1, compare_op=Alu.is_equal, fill=0.0,
            )
            # causal mask in [j (part), t (free)] layout: keep j <= t
            maskT = consts.tile([128, 128], f32)
            nc.gpsimd.affine_select(
                maskT[:], ones[:], pattern=[[1, 128]], base=0,
                channel_multiplier=-1, compare_op=Alu.is_ge, fill=0.0,
            )

            # ---- weights: [D, HDC] -> [128, DC, HDC] (partition = d % 128) ----
            w_sb = {}
            for name, wdram in (("q", wq), ("k", wk), ("v", wv)):
                wt = wpool.tile([128, DC, HDC], f32r, name=f"w{name}_sb")
                nc.sync.dma_start(wt[:], wdram.rearrange("(dc p) m -> p dc m", p=128))
                w_sb[name] = wt

            # ---- resident activations ----
            # head pairs hp=0 (heads 0,1) / hp=1 (heads 2,3), head at partition 64*(h%2)
            phi_qT = [res.tile([128, S], f32, name=f"phi_qT{i}") for i in range(2)]
            phi_kT = [res.tile([128, S], f32, name=f"phi_kT{i}") for i in range(2)]
            # seq-major: [s-in-chunk, (chunk, head, :)]
            phi_ks = res.tile([128, NCHUNK * HDC], f32, name="phi_ks")
            v_aug = res.tile([128, NCHUNK * HPC * 65], f32, name="v_aug")
            # ones column of v_aug (written once; v copies fill the rest)
            nc.vector.memset(v_aug.rearrange("p (c h e) -> p c h e", c=NCHUNK, h=HPC)[:, :, :, 64:65], 1.0)

            # ---- load x^T tiles (streamed by s-half to bound SBUF) ----
            def load_half(xdram, qt, tag):
                tiles = []
                for dc in range(DC):
                    t = xin.tile([128, S // 4], f32r, name=f"x_{tag}_{qt}_{dc}", tag="xin")
                    nc.sync.dma_start(t[:], xdram[dc * 128:(dc + 1) * 128, qt * (S // 4):(qt + 1) * (S // 4)])
                    tiles.append(t)
                return tiles

            def phi_from_psum(ps, dst, n):
                # phi(x) = exp(min(x,0)) + max(x,0); m' = relu(-x); e = exp(-m')
                t1 = work.tile([128, n], f32, tag="phi1")
                nc.scalar.activation(t1[:], ps[:], Act.Relu, scale=-1.0)
                t2 = work.tile([128, n], f32, tag="phi2")
                nc.scalar.activation(t2[:], t1[:], Act.Exp, scale=-1.0)
                nc.vector.scalar_tensor_tensor(
                    dst, ps[:], 0.0, t2[:], op0=Alu.max, op1=Alu.add)

            # ---- per-half pipeline: proj q/k/v -> phi_k transposes -> attention ----
            S_prev = []
            for hp in range(2):
                s0t = res.tile([128, 65], f32, name=f"S_init{hp}")
                nc.vector.memset(s0t[:], 0.0)
                S_prev.append(s0t)
            vaug4 = v_aug.rearrange("p (c h e) -> p c h e", c=NCHUNK, h=HPC)

            for qt in range(4):
                # q/k projections for this quarter (one 512-wide col group)
                for tname, dst in (("q", phi_qT), ("k", phi_kT)):
                    xt = load_half({"q": xqT, "k": xkT}[tname], qt, tname)
                    for hp in range(2):
                        s0 = qt * 512
                        ps = psum.tile([128, 512], f32, tag="proj", name=f"ps_{tname}_{qt}_{hp}")
                        for dc in range(DC):
                            nc.tensor.matmul(
                                ps[:], w_sb[tname][:, dc, hp * 128:(hp + 1) * 128],
                                xt[dc][:],
                                start=(dc == 0), stop=(dc == DC - 1),
                            )
                        phi_from_psum(ps, dst[hp][:, s0:s0 + 512], 512)

                # v projection for this quarter (seq-major)
                xt = load_half(xvT, qt, "v")
                for cc in range(NCHUNK // 4):
                    c = qt * (NCHUNK // 4) + cc
                    ps = psum.tile([128, HDC], f32, tag="proj", name=f"ps_v_{c}")
                    for dc in range(DC):
                        nc.tensor.matmul(
                            ps[:], xt[dc][:, cc * 128:(cc + 1) * 128],
                            w_sb["v"][:, dc, :],
                            start=(dc == 0), stop=(dc == DC - 1),
                        )
                    dstv = vaug4[:, c, :, 0:64]
                    nc.any.tensor_copy(dstv, ps.rearrange("p (h e) -> p h e", h=HPC)[:])

                # phi_k seq-major via PE transpose (this quarter's chunks)
                for cc in range(NCHUNK // 4):
                    c = qt * (NCHUNK // 4) + cc
                    for h in range(HPC):
                        hp, hb = h // 2, 64 * (h % 2)
                        tp = psum.tile([128, 64], f32, tag="proj", bufs=2, name=f"tp_{c}_{h}")
                        nc.tensor.transpose(
                            tp[:], phi_kT[hp][hb:hb + 64, c * 128:(c + 1) * 128],
                            ident[hb:hb + 64, hb:hb + 64],
                        )
                        nc.any.tensor_copy(phi_ks[:, c * HDC + h * 64: c * HDC + (h + 1) * 64], tp[:])

                # attention for this quarter's chunks
                for cc in range(NCHUNK // 4):
                    c = qt * (NCHUNK // 4) + cc
                    o_ps = {}
                    for h in range(HPC):
                        hp, hb = h // 2, 64 * (h % 2)
                        kT_c = phi_kT[hp][hb:hb + 64, c * 128:(c + 1) * 128]
                        qT_c = phi_qT[hp][hb:hb + 64, c * 128:(c + 1) * 128]
                        a_ps = psum.tile([128, 128], f32, tag="A", name=f"a_ps_{c}_{h}")
                        nc.tensor.matmul(a_ps[:], kT_c, qT_c, start=True, stop=True)
                        a_sb = attn.tile([128, 128], f32, tag="Asb", name=f"a_sb_{c}_{h}", bufs=6)
                        nc.vector.tensor_tensor(a_sb[:], a_ps[:], maskT[:], op=Alu.mult)
                        op = psum.tile([128, 65], f32, tag="o", name=f"o_ps_{c}_{h}")
                        nc.tensor.matmul(op[:], a_sb[:], vaug4[:, c, h, :],
                                         start=True, stop=(c == 0))
                        if c > 0:
                            nc.tensor.matmul(op[:], qT_c, S_prev[hp][hb:hb + 64, :],
                                             start=False, stop=True)
                        o_ps[h] = op

                    S_new = []
                    for hp in range(2):
                        s_inc = psum.tile([128, 130], f32, tag="Sinc", name=f"s_inc_{c}_{hp}")
                        nc.tensor.matmul(
                            s_inc[:],
                            phi_ks[:, c * HDC + hp * 128: c * HDC + (hp + 1) * 128],
                            vaug4[:, c, 2 * hp:2 * hp + 2, :],
                            start=True, stop=True,
                        )
                        sn = attn.tile([128, 65], f32, tag=f"S{hp}", name=f"S_{c}_{hp}", bufs=2)
                        nc.vector.tensor_tensor(sn[0:64, :], S_prev[hp][0:64, :], s_inc[0:64, 0:65], op=Alu.add)
                        nc.vector.tensor_tensor(sn[64:128, :], S_prev[hp][64:128, :], s_inc[64:128, 65:130], op=Alu.add)
                        S_new.append(sn)
                    S_prev = S_new

                    o_sb = attn.tile([128, HDC], f32, tag="osb", name=f"o_sb_{c}")
                    for h in range(HPC):
                        op = o_ps[h]
                        den = attn.tile([128, 1], f32, tag="den", name=f"den_{c}_{h}", bufs=4)
                        nc.vector.tensor_scalar(den[:], op[:, 64:65], EPS, None, op0=Alu.add)
                        rcp = attn.tile([128, 1], f32, tag="rcp", name=f"rcp_{c}_{h}", bufs=4)
                        nc.vector.reciprocal(rcp[:], den[:])
                        nc.vector.tensor_scalar(o_sb[:, h * 64:(h + 1) * 64], op[:, 0:64],
                                                rcp[:], None, op0=Alu.mult)
                    nc.sync.dma_start(out[c * 128:(c + 1) * 128, :], o_sb[:])

            if dump:
                for i in range(2):
                    nc.sync.dma_start(d_phi_qT[i][:], phi_qT[i][:])
                    nc.sync.dma_start(d_phi_kT[i][:], phi_kT[i][:])
                nc.sync.dma_start(d_phi_ks[:], phi_ks[:])
                nc.sync.dma_start(d_v_aug[:], v_aug[:])

    nc.compile()
    return nc


def _get_nc():
    with _lock:
        if "nc" not in _cache:
            _cache["nc"] = _build_nc()
        return _cache["nc"]


def kernel(query, key, value, query_kernel, key_kernel, value_kernel):
    from concourse.bass_utils import run_bass_kernel_spmd

    nc = _get_nc()

    xT = {}
    for b in range(B):
        xT[("q", b)] = np.ascontiguousarray(query[b].T, dtype=np.float32)
        xT[("k", b)] = np.ascontiguousarray(key[b].T, dtype=np.float32)
        xT[("v", b)] = np.ascontiguousarray(value[b].T, dtype=np.float32)

    in_maps = []
    for c in range(N_CORES):
        b, h0 = c // 4, 4 * (c % 4)
        in_maps.append({
            "xqT": xT[("q", b)],
            "xkT": xT[("k", b)],
            "xvT": xT[("v", b)],
            "wq": np.ascontiguousarray(query_kernel[:, h0:h0 + HPC, :].reshape(D, HDC), dtype=np.float32),
            "wk": np.ascontiguousarray(key_kernel[:, h0:h0 + HPC, :].reshape(D, HDC), dtype=np.float32),
            "wv": np.ascontiguousarray(value_kernel[:, h0:h0 + HPC, :].reshape(D, HDC), dtype=np.float32),
        })

    results = run_bass_kernel_spmd(nc, in_maps, core_ids=list(range(N_CORES)))

    # The reference ends with a FLAT reshape of [B*H, S, HD] -> (B, S, H*HD):
    # output rows [128h:128h+128] of batch b are head h's [S, HD] attention
    # output flat-reshaped to [128, H*HD].
    full = np.empty((B, S, H * HD), dtype=np.float32)
    for c in range(N_CORES):
        b, h0 = c // 4, 4 * (c % 4)
        av = results.results[c]["out"].reshape(S, HPC, HD)
        for hl in range(HPC):
            full[b, (h0 + hl) * 128:(h0 + hl + 1) * 128, :] = (
                av[:, hl, :].reshape(128, H * HD))
    return full



# revision 43
# speedup vs baseline: 1.5432x; 1.5432x over previous
"""Causal linear attention (elu+1 feature map) for Trainium2, 8 NeuronCores.

Problem: B=2, S=2048, D=1024, H=16, HD=64.
  q/k/v projections [S,D]@[D,H*HD], phi = elu+1, causal linear attention
  out[t] = (sum_{i<=t} (phi_q[t].phi_k[i]) v[i]) / (phi_q[t].sum_{i<=t} phi_k[i] + eps)

Sharding: core c -> (batch b=c//4, heads h0=4*(c%4) .. h0+3). No cross-core comm.
Host feeds x^T [D,S] per core in bf16 so the contraction dim d sits on SBUF
partitions with no on-chip transposes, at half the HBM traffic of fp32.

Device algorithm (per core, 4 heads; bf16 matmul inputs, fp32 PSUM):
  - warmup matmuls on constants ramp the PE p-state while the first DMAs land
  - proj q,k -> phi_qT/phi_kT [64,2048] bf16 per head (head-pairs on 128 parts)
  - phi_k seq layout via PE transpose (bf16, 4 heads share one PSUM tile)
  - v projected seq-major with an appended ones column (v_aug, 65 cols/head)
  - chunked attention, L=128: A4 = one PSUM tile with all 4 heads' k^T q;
    one DVE op masks all 4 heads (j<=t) -> bf16; per head
    out_psum[:,h*65:+65] = tril(A)^T-form @ v_aug + phi_q @ S_prev.
    S state accumulates in a persistent PSUM tile (matmul start only on c==0),
    copied to bf16 SBUF once per chunk for the phi_q @ S_prev matmul.
    The ones column of v_aug makes column 64 of each head's slice the
    normalizer; one [128,4] reciprocal scales all 4 heads.
"""

import threading

import numpy as np

B, S, D, H, HD = 2, 2048, 1024, 16, 64
EPS = 1e-6
N_CORES = 8
HPC = 4            # heads per core
HDC = HPC * HD     # 256 projected cols per core
NCHUNK = S // 128  # 16
DC = D // 128      # 8 contraction chunks
QT = 4             # seq quarters for the load/proj pipeline
SQ = S // QT       # 512
CPQ = NCHUNK // QT  # 4 chunks per quarter
WARMUP = 20        # PE p-state warmup matmuls

_lock = threading.Lock()
_cache = {}


def _build_nc():
    import concourse.tile as tile
    from concourse import bacc, mybir

    f32 = mybir.dt.float32
    bf16 = mybir.dt.bfloat16
    Alu = mybir.AluOpType
    Act = mybir.ActivationFunctionType

    nc = bacc.Bacc("TRN2", target_bir_lowering=False, debug=False)

    # x is host-packed as [p, qt, dc, s'] = x[qt*SQ+s', dc*128+p] so one
    # quarter-load is 128 descriptors of 8KB contiguous DRAM each; w is
    # host-packed as [p, dc, m] = w[dc*128+p, m] (one 4KB run per partition).
    xdram = {}
    wdram = {}
    for name, tgt in (("q", "xq"), ("k", "xk"), ("v", "xv")):
        xdram[name] = nc.dram_tensor(tgt, [128, QT, DC, SQ], bf16, kind="ExternalInput").ap()
        wdram[name] = nc.dram_tensor("w" + name, [128, DC, HDC], bf16, kind="ExternalInput").ap()
    out = nc.dram_tensor("out", [S, HDC], bf16, kind="ExternalOutput").ap()

    with tile.TileContext(nc) as tc:
        with (
            tc.tile_pool(name="consts", bufs=1) as consts,
            tc.tile_pool(name="weights", bufs=1) as wpool,
            tc.tile_pool(name="res", bufs=1) as res,
            tc.tile_pool(name="xin", bufs=3) as xin,
            tc.tile_pool(name="work", bufs=3) as work,
            tc.tile_pool(name="attn", bufs=3) as attn,
            tc.tile_pool(name="psum", bufs=2, space="PSUM") as psum,
        ):
            # ---- constants ----
            # ones comes first via a fast DVE memset so warmup matmuls can
            # start ~0.5us in, without waiting for the Pool affine_selects
            ones = consts.tile([128, 512], bf16)
            nc.vector.memset(ones[:], 1.0)
            ident = consts.tile([128, 128], bf16)
            nc.gpsimd.affine_select(
                ident[:], ones[:, 0:128], pattern=[[-1, 128]], base=0,
                channel_multiplier=1, compare_op=Alu.is_equal, fill=0.0,
            )
            # causal mask in [j (part), t (free)] layout: keep j <= t; 4 copies
            maskT4 = consts.tile([128, 512], bf16)
            for h in range(HPC):
                nc.gpsimd.affine_select(
                    maskT4[:, h * 128:(h + 1) * 128], ones[:, 0:128],
                    pattern=[[1, 128]], base=0,
                    channel_multiplier=-1, compare_op=Alu.is_ge, fill=0.0,
                )

            # ---- weights ----
            w_sb = {}
            for name in ("q", "k", "v"):
                w_sb[name] = wpool.tile([128, DC, HDC], bf16, name=f"w{name}_sb")
            nc.sync.dma_start(w_sb["q"][:], wdram["q"][:])

            # ---- PE p-state warmup: dense back-to-back matmuls on constants.
            # They write the rotating "proj" psum slots (never read) and keep
            # the tensor engine busy while the first x/w DMAs land, so real
            # matmuls start at full clock.
            for i in range(WARMUP):
                wps = psum.tile([128, 512], f32, tag="proj", name=f"warm_{i}")
                nc.tensor.matmul(wps[:], ones[:, 0:128], ones[:], start=True, stop=True)

            # ---- resident activations ----
            # head pairs hp=0 (heads 0,1) / hp=1 (heads 2,3), head at partition 64*(h%2)
            phi_qT = [res.tile([128, S], bf16, name=f"phi_qT{i}") for i in range(2)]
            phi_kT = [res.tile([128, S], bf16, name=f"phi_kT{i}") for i in range(2)]
            # seq-major: [s-in-chunk, (chunk, head, :)]
            phi_ks = res.tile([128, NCHUNK * HDC], bf16, name="phi_ks")
            v_aug = res.tile([128, NCHUNK * HPC * 65], bf16, name="v_aug")
            vaug4 = v_aug.rearrange("p (c h e) -> p c h e", c=NCHUNK, h=HPC)
            nc.vector.memset(vaug4[:, :, :, 64:65], 1.0)
            # bf16 copy of the running state for the phi_q @ S_prev matmul
            S_sb = [None, None]

            def load_x(name, qt):
                t = xin.tile([128, DC, SQ], bf16, name=f"x_{name}_{qt}", tag=f"x{name}")
                nc.sync.dma_start(t[:], xdram[name][:, qt])
                return t

            def load_quarter(qt, tiles=None):
                tiles = {}
                for name in ("q", "k", "v"):
                    tiles[name] = load_x(name, qt)
                return tiles

            def phi_from_psum(ps, dst, n, tag):
                # phi(x) = exp(min(x,0)) + max(x,0); m' = relu(-x); e = exp(-m')
                t1 = work.tile([128, n], f32, tag=f"{tag}1", name=f"t1_{tag}")
                nc.scalar.activation(t1[:], ps[:], Act.Relu, scale=-1.0)
                t2 = work.tile([128, n], f32, tag=f"{tag}2", name=f"t2_{tag}")
                nc.scalar.activation(t2[:], t1[:], Act.Exp, scale=-1.0)
                nc.vector.scalar_tensor_tensor(
                    dst, ps[:], 0.0, t2[:], op0=Alu.max, op1=Alu.add)

            def emit_proj_phi(qt, xt, tname, hp):
                # one 128-projdim x 512-seq group: 8 accumulating matmuls + phi
                dst = {"q": phi_qT, "k": phi_kT}[tname]
                ps = psum.tile([128, SQ], f32, tag="proj", name=f"ps_{tname}_{qt}_{hp}")
                for dc in range(DC):
                    nc.tensor.matmul(
                        ps[:], w_sb[tname][:, dc, hp * 128:(hp + 1) * 128],
                        xt[tname][:, dc, :],
                        start=(dc == 0), stop=(dc == DC - 1),
                    )
                phi_from_psum(ps, dst[hp][:, qt * SQ:qt * SQ + SQ], SQ, "phi")

            def emit_vproj(qt, cc, xt):
                c = qt * CPQ + cc
                ps = psum.tile([128, HDC], f32, tag="vproj", name=f"ps_v_{c}", bufs=1)
                for dc in range(DC):
                    nc.tensor.matmul(
                        ps[:], xt["v"][:, dc, cc * 128:(cc + 1) * 128],
                        w_sb["v"][:, dc, :],
                        start=(dc == 0), stop=(dc == DC - 1),
                    )
                nc.scalar.activation(
                    vaug4[:, c, :, 0:64],
                    ps.rearrange("p (h e) -> p h e", h=HPC), Act.Copy)

            def emit_transp(c):
                # phi_k seq-major via PE transpose (bf16 psum): one full
                # [128,128] transpose per head-pair. NB hardware rejects psum
                # banks shared by matmuls with differing partition offsets;
                # full-width (offset-0) writes can share, so the tp bank also
                # hosts the hb=0 A matmuls (tag A0).
                tp = psum.tile([128, HDC], bf16, tag="A0", name=f"tp_{c}", bufs=1)
                for hp in range(2):
                    nc.tensor.transpose(
                        tp[:, hp * 128:(hp + 1) * 128],
                        phi_kT[hp][:, c * 128:(c + 1) * 128],
                        ident[:],
                    )
                nc.scalar.activation(
                    phi_ks[:, c * HDC:(c + 1) * HDC], tp[:], Act.Copy)

            a_sbs = {}

            def emit_stageA(c):
                # A matmuls parity-split over two psum banks: heads {0,2}
                # read partitions 0:64 (bank A0), heads {1,3} partitions
                # 64:128 (bank A1) — a bank must not mix partition offsets.
                # a_sb column block for head h: 256*(h%2) + 128*(h//2).
                a_sb = attn.tile([128, 512], bf16, tag="Asb", name=f"a_sb_{c}")
                for par in range(2):
                    a_ps = psum.tile([128, 256], f32, tag=f"A{par}",
                                     name=f"a_ps_{c}_{par}", bufs=1)
                    hb = 64 * par
                    for j, h in enumerate((par, par + 2)):
                        hp = h // 2
                        nc.tensor.matmul(
                            a_ps[:, j * 128:(j + 1) * 128],
                            phi_kT[hp][hb:hb + 64, c * 128:(c + 1) * 128],
                            phi_qT[hp][hb:hb + 64, c * 128:(c + 1) * 128],
                            start=(j == 0), stop=(j == 1))
                    nc.vector.tensor_tensor(
                        a_sb[:, par * 256:(par + 1) * 256], a_ps[:],
                        maskT4[:, 0:256], op=Alu.mult)
                a_sbs[c] = a_sb

                for hp in range(2):
                    nc.tensor.matmul(
                        S_ps[:, hp * 130:(hp + 1) * 130],
                        phi_ks[:, c * HDC + hp * 128:c * HDC + (hp + 1) * 128],
                        vaug4[:, c, 2 * hp:2 * hp + 2, :].rearrange("p h e -> p (h e)"),
                        start=(c == 0 and hp == 0),
                        stop=(c == NCHUNK - 1 and hp == 1),
                        skip_group_check=True,
                    )
                sn = attn.tile([128, 260], bf16, tag="Ssb", name=f"S_{c}", bufs=3)
                nc.scalar.activation(sn[:], S_ps[:], Act.Copy)
                S_sb[c % 2] = sn

            def emit_stageO(c):
                # o matmuls parity-split like A: bank o<par> holds heads
                # (par, par+2) at cols 0:65 / 65:130. The 128-part A@v may mix
                # with either hb's q@S in a bank, but hb=0 and hb=64 may not
                # share one.
                a_sb = a_sbs.pop(c)
                o_pss = []
                for par in range(2):
                    o_ps = psum.tile([128, 130], f32, tag=f"o{par}",
                                     name=f"o_ps_{c}_{par}", bufs=1)
                    o_pss.append(o_ps)
                    hb = 64 * par
                    for j, h in enumerate((par, par + 2)):
                        hp = h // 2
                        if c > 0:
                            nc.tensor.matmul(
                                o_ps[:, j * 65:j * 65 + 65],
                                phi_qT[hp][hb:hb + 64, c * 128:(c + 1) * 128],
                                S_sb[(c - 1) % 2][hb:hb + 64, hp * 130 + 65 * par:hp * 130 + 65 * par + 65],
                                start=(j == 0), stop=False)
                        nc.tensor.matmul(
                            o_ps[:, j * 65:j * 65 + 65],
                            a_sb[:, par * 256 + hp * 128:par * 256 + (hp + 1) * 128],
                            vaug4[:, c, h, :],
                            start=(c == 0 and j == 0), stop=(j == 1))

                # normalize: per-parity [128,2] eps-add + reciprocal, then 4
                # scaled copies split across DVE and Act (both are chain-bound
                # in the serial tail)
                # one slot per chunk for o_sb: output DMAs queue behind
                # multi-us x-loads and must never backpressure compute
                o_sb = attn.tile([128, HDC], bf16, tag="osb", name=f"o_sb_{c}", bufs=NCHUNK)
                for par in range(2):
                    o2 = o_pss[par].rearrange("p (j e) -> p j e", j=2)
                    den = attn.tile([128, 2], f32, tag=f"den{par}", name=f"den_{c}_{par}", bufs=4)
                    nc.vector.tensor_scalar(den[:], o2[:, :, 64], EPS, None, op0=Alu.add)
                    rcp = attn.tile([128, 2], f32, tag=f"rcp{par}", name=f"rcp_{c}_{par}", bufs=4)
                    nc.vector.reciprocal(rcp[:], den[:])
                    for j, h in enumerate((par, par + 2)):
                        if par == 0:
                            nc.vector.tensor_scalar(
                                o_sb[:, h * 64:(h + 1) * 64], o2[:, j, 0:64],
                                rcp[:, j:j + 1], None, op0=Alu.mult)
                        else:
                            nc.scalar.activation(
                                o_sb[:, h * 64:(h + 1) * 64], o2[:, j, 0:64],
                                Act.Copy, scale=rcp[:, j:j + 1])
                # SWDGE store: Pool frees its sequencer before the transfer,
                # so output DMAs never head-of-line block the input loads.
                OUT_DMA(out[c * 128:(c + 1) * 128, :], o_sb[:])

            OUT_DMA = nc.sync.dma_start
            S_ps = psum.tile([128, 260], f32, tag="S", name="S_ps", bufs=1)

            # ---- software-pipelined emission: cursor-pumped streams.
            # Streams (vproj -> transpose -> stageA -> stageO) advance one item
            # per "pump", each lagging its producer by >= 1 pump so every
            # cross-engine dependency is about a pump (~1.5-2us of PE work) old
            # by the time the PE queue reaches the consumer. stageO(c) is
            # emitted before stageA(c+1) so the shared A-bank WAR never points
            # forward in the PE program order.
            st = {"p": 0, "tp": 0, "aa": 0, "oo": 0, "vp": 0}
            k1_pump = {}
            vp_pump = {}
            tp_pump = {}
            aa_pump = {}

            import os as _os
            _LVL = int(_os.environ.get("KBISECT", "3"))  # 0=proj only,1=+transp,2=+stageA,3=full

            def pump():
                st["p"] += 1
                p = st["p"]
                if _LVL >= 3 and st["oo"] < st["aa"] and aa_pump[st["oo"]] < p:
                    emit_stageO(st["oo"])
                    st["oo"] += 1
                c = st["tp"]
                if (_LVL >= 1 and c < st["vp"] and c // CPQ in k1_pump
                        and p - k1_pump[c // CPQ] >= 2 and p - vp_pump[c] >= 1):
                    emit_transp(c)
                    tp_pump[c] = p
                    st["tp"] += 1
                c = st["aa"]
                if _LVL >= 2 and c < st["tp"] and tp_pump[c] < p:
                    emit_stageA(c)
                    aa_pump[c] = p
                    st["aa"] += 1

            xt = {"q": load_x("q", 0)}
            nc.sync.dma_start(w_sb["k"][:], wdram["k"][:])
            xt["k"] = load_x("k", 0)
            nc.sync.dma_start(w_sb["v"][:], wdram["v"][:])
            xt["v"] = load_x("v", 0)
            for qt in range(QT):
                if qt > 0:
                    xt = load_quarter(qt)
                for gi, (tname, hp) in enumerate((("q", 0), ("q", 1), ("k", 0), ("k", 1))):
                    emit_proj_phi(qt, xt, tname, hp)
                    if gi == 3:
                        k1_pump[qt] = st["p"] + 1
                    pump()
                for cc in range(CPQ):
                    emit_vproj(qt, cc, xt)
                    vp_pump[qt * CPQ + cc] = st["p"] + 1
                    st["vp"] += 1
                    pump()
            if _LVL >= 3:
                while st["oo"] < NCHUNK:
                    pump()
            elif _LVL >= 1:
                while st["tp"] < NCHUNK or (_LVL >= 2 and st["aa"] < NCHUNK):
                    pump()
            if _LVL < 3:
                # dummy stores so the output is produced
                for c in range(NCHUNK):
                    dummy = attn.tile([128, HDC], bf16, tag="osb", name=f"dummy_{c}", bufs=2)
                    nc.vector.tensor_copy(dummy[:], v_aug[:, c * HDC:(c + 1) * HDC])
                    nc.sync.dma_start(out[c * 128:(c + 1) * 128, :], dummy[:])

    nc.compile()
    return nc


def _get_nc():
    with _lock:
        if "nc" not in _cache:
            _cache["nc"] = _build_nc()
        return _cache["nc"]


def kernel(query, key, value, query_kernel, key_kernel, value_kernel):
    import ml_dtypes
    from concourse.bass_utils import run_bass_kernel_spmd

    bf16 = ml_dtypes.bfloat16
    nc = _get_nc()

    def pack_x(x):
        # [S, D] -> [p, qt, dc, s'] with s = qt*SQ+s', d = dc*128+p
        return np.ascontiguousarray(
            x.astype(bf16).reshape(QT, SQ, DC, 128).transpose(3, 0, 2, 1))

    def pack_w(w, h0):
        # [D, H, HD] slice -> [p, dc, m] with d = dc*128+p
        return np.ascontiguousarray(
            w[:, h0:h0 + HPC, :].reshape(DC, 128, HDC).transpose(1, 0, 2).astype(bf16))

    xT = {}
    for b in range(B):
        xT[("q", b)] = pack_x(query[b])
        xT[("k", b)] = pack_x(key[b])
        xT[("v", b)] = pack_x(value[b])

    in_maps = []
    for c in range(N_CORES):
        b, h0 = c // 4, 4 * (c % 4)
        in_maps.append({
            "xq": xT[("q", b)],
            "xk": xT[("k", b)],
            "xv": xT[("v", b)],
            "wq": pack_w(query_kernel, h0),
            "wk": pack_w(key_kernel, h0),
            "wv": pack_w(value_kernel, h0),
        })

    results = run_bass_kernel_spmd(nc, in_maps, core_ids=list(range(N_CORES)))

    # The reference ends with a FLAT reshape of [B*H, S, HD] -> (B, S, H*HD):
    # output rows [128h:128h+128] of batch b are head h's [S, HD] attention
    # output flat-reshaped to [128, H*HD].
    full = np.empty((B, S, H * HD), dtype=np.float32)
    for c in range(N_CORES):
        b, h0 = c // 4, 4 * (c % 4)
        av = results.results[c]["out"].astype(np.float32).reshape(S, HPC, HD)
        for hl in range(HPC):
            full[b, (h0 + hl) * 128:(h0 + hl + 1) * 128, :] = (
                av[:, hl, :].reshape(128, H * HD))
    return full
